# revision 13
# baseline (speedup 1.0000x reference)
"""Sparse-attention Trainium2 kernel (nn_AttentionLayer, B=16 S=2048 D=128).

reference semantics:
    A = Q @ T^T                     # [B,S,S]
    A = where(A > 0.3, A, 0)
    A += where(strictly_upper, -2^32, 0)
    y = softmax(A / sqrt(D)) @ V

Sharding: data-parallel over batch, 2 batches per core on 8 NeuronCores.

v4: permuted contiguous loads.
  All inputs are loaded with per-partition-contiguous DMA patterns so
  triggers are cheap and transfers fast:
    Q[b]: slab s (512 rows), q = 512s + 4p + jq   -> [128p, 4jq, 128d]
    T[b], V[b]: half h (1024 rows), k = 512K + 4p + jj
                                               -> [128p, (K jj)=8, 128d]
  A "k-chunk" c (=4K+jj) holds keys {512K + 4p + jj : p}. Scores are
  computed transposed S^T[k-chunk, q] with q columns enumerated
  p-major so queries appear in NATURAL order (q = 512*qb + col).
  Chunks with K == qb straddle the diagonal; their causal mask is a
  4-row staircase R01[jj][pk, col] = col >= 4*pk + jj, applied as a
  bf16 multiply after exp+max (masked garbage scores are zeroed then).

  num = max(exp(S^T*scale),1): ScalarE exp [128,1024] per group
  (fp32 PSUM -> bf16 SBUF), VectorE tensor_scalar_max (4x mode).
  PV + denominator per (chunk, q-subtile): lhsT = num chunk, rhs =
  [V | ones] [128k,129], PSUM-accumulated. out = PV/den via VectorE
  PSUM->SBUF copy + GpSimd normalize_recip. Output store is natural.

  Queue discipline (DMA queues are FIFO; any dep on a DMA waits for
  ALL earlier DMAs on that queue):
    scalar queue: batch-0 loads only (8 cheap triggers, done ~11us,
      ScalarE then runs exp undisturbed).
    sync queue: xbar transposes first (nothing bulky ahead of them),
      then batch-1 whole-tensor loads, then output stores.
"""

from collections import deque
from contextlib import ExitStack

import numpy as np

import concourse.bass as bass
import concourse.mybir as mybir
import concourse.tile as tile
from concourse import bacc

B, S, D = 16, 2048, 128
N_CORES = 8
B_LOC = B // N_CORES
QB = 512
N_QB = S // QB
SCALE = float(1.0 / np.sqrt(D))

F32 = mybir.dt.float32
BF16 = mybir.dt.bfloat16
Alu = mybir.AluOpType


def build_attention_core():
    nc = bacc.Bacc("TRN2", target_bir_lowering=False, debug=False,
                   num_devices=N_CORES)
    q_ext = nc.dram_tensor("Q", [B_LOC, S, D], F32, kind="ExternalInput").ap()
    t_ext = nc.dram_tensor("T", [B_LOC, S, D], F32, kind="ExternalInput").ap()
    v_ext = nc.dram_tensor("V", [B_LOC, S, D], F32, kind="ExternalInput").ap()
    o_ext = nc.dram_tensor("out", [B_LOC, S, D], F32, kind="ExternalOutput").ap()

    with tile.TileContext(nc) as tc, ExitStack() as ctx:
        const_pool = ctx.enter_context(tc.tile_pool(name="const", bufs=1))
        nat_pool = ctx.enter_context(tc.tile_pool(name="nat", bufs=1))
        stage_pool = ctx.enter_context(tc.tile_pool(name="stage", bufs=1))
        tpd_pool = ctx.enter_context(tc.tile_pool(name="tpd", bufs=1))
        vb_pool = ctx.enter_context(tc.tile_pool(name="vb", bufs=1))
        num_pool = ctx.enter_context(tc.tile_pool(name="num", bufs=6))
        fin_pool = ctx.enter_context(tc.tile_pool(name="fin", bufs=3))
        rec_pool = ctx.enter_context(tc.tile_pool(name="rec", bufs=4))
        qk_psum = ctx.enter_context(tc.tile_pool(name="qk_ps", bufs=2, space="PSUM"))
        ob_psum = ctx.enter_context(tc.tile_pool(name="ob_ps", bufs=4, space="PSUM"))

        # ---- constants (gpsimd) ----
        junk = const_pool.tile([128, 512], BF16, name="junk")
        nc.gpsimd.memset(junk[:], 0.25)
        # r01p[i][pk, 512*j + col] = 1 if col >= 4*pk + (2i+j) else 0 —
        # the diag keep-masks for a group's chunk pair (jj = 2i, 2i+1)
        r01p = []
        for i in range(2):
            m = const_pool.tile([128, 2, 512], BF16, name=f"r01p_{i}")
            nc.gpsimd.memset(m[:], 1.0)
            for j in range(2):
                nc.gpsimd.affine_select(
                    out=m[:, j, :], in_=m[:, j, :],
                    compare_op=Alu.is_ge, fill=0.0,
                    base=-(2 * i + j), channel_multiplier=-4,
                    pattern=[[1, 512]])
            r01p.append(m)

        # ---- staging tensors ----
        # batch-0: per-piece nat tensors (own DMA each); batch-1: whole.
        qnat0 = [nat_pool.tile([128, 4, 128], F32, name=f"qn0s{s}")
                 for s in range(4)]
        tnat0 = [nat_pool.tile([128, 8, 128], F32, name=f"tn0h{h}")
                 for h in range(2)]
        vnat0 = [nat_pool.tile([128, 8, 128], F32, name=f"vn0h{h}")
                 for h in range(2)]
        nat1 = {w: nat_pool.tile([128, 16, 128], F32, name=f"n1{w}")
                for w in ("q", "t", "v")}

        qstg = [[stage_pool.tile([128, 4, 128], BF16, name=f"qs{b}s{s}")
                 for s in range(4)] for b in range(2)]
        tstg = [[stage_pool.tile([128, 8, 128], BF16, name=f"ts{b}h{h}")
                 for h in range(2)] for b in range(2)]
        qtp = [[tpd_pool.tile([128, 4, 128], BF16, name=f"qt{b}s{s}")
                for s in range(4)] for b in range(2)]
        ttp = [[tpd_pool.tile([128, 8, 128], BF16, name=f"tt{b}h{h}")
                for h in range(2)] for b in range(2)]
        vaug = [[vb_pool.tile([128, 8, 129], BF16, name=f"va{b}h{h}")
                 for h in range(2)] for b in range(2)]
        for b in range(2):
            for h in range(2):
                nc.gpsimd.memset(vaug[b][h][:, :, D:D + 1], 1.0)

        # ---- load helpers ----
        def load_q0(s):
            nc.scalar.dma_start(
                qnat0[s][:],
                q_ext[0, 512 * s:512 * (s + 1), :]
                .rearrange("(p j) d -> p j d", p=128))

        def load_th0(which, h):
            ext = t_ext if which == "t" else v_ext
            dst = tnat0[h] if which == "t" else vnat0[h]
            nc.scalar.dma_start(
                dst[:].rearrange("p (K j) d -> p K j d", K=2),
                ext[0, 1024 * h:1024 * (h + 1), :]
                .rearrange("(K p j) d -> p K j d", p=128, j=4))

        def load_b1(which):
            ext = {"q": q_ext, "t": t_ext, "v": v_ext}[which]
            nc.sync.dma_start(
                nat1[which][:].rearrange("p (K j) d -> p K j d", K=4),
                ext[1].rearrange("(K p j) d -> p K j d", p=128, j=4))

        # batch-1 nat slot ranges: q slab s -> [4s:4s+4); t/v half h -> [8h:8h+8)
        def cast_q(b, s):
            src = qnat0[s][:] if b == 0 else nat1["q"][:, 4 * s:4 * s + 4, :]
            nc.vector.tensor_copy(qstg[b][s][:], src)

        def cast_t(b, h):
            src = tnat0[h][:] if b == 0 else nat1["t"][:, 8 * h:8 * h + 8, :]
            nc.vector.tensor_copy(tstg[b][h][:], src)

        def cast_v(b, h):
            src = vnat0[h][:] if b == 0 else nat1["v"][:, 8 * h:8 * h + 8, :]
            nc.vector.tensor_copy(vaug[b][h][:, :, 0:D], src)

        def xpose_q(b, s):
            nc.sync.dma_start_transpose(
                qtp[b][s][:], qstg[b][s][:].rearrange("p j d -> p (j d)"))

        def xpose_t(b, h):
            nc.sync.dma_start_transpose(
                ttp[b][h][:], tstg[b][h][:].rearrange("p c d -> p (c d)"))

        # ---- PE warm-up: ramp the p-state while DMA prep runs ----
        for w in range(8):
            wps = qk_psum.tile([128, 1024], F32, tag="qk", name=f"wps{w}")
            nc.tensor.matmul(wps[:, 0:512], lhsT=junk[:, 0:128], rhs=junk[:])

        # ---- batch-0 loads on the scalar queue, critical-first ----
        load_th0("t", 0)
        load_q0(0)
        load_th0("v", 0)
        load_q0(1)
        load_th0("t", 1)
        load_q0(2)
        load_q0(3)
        load_th0("v", 1)

        cast_t(0, 0)
        cast_q(0, 0)
        cast_v(0, 0)
        cast_q(0, 1)
        cast_t(0, 1)
        cast_q(0, 2)
        cast_q(0, 3)
        cast_v(0, 1)

        # sync queue: transposes first (FIFO kept clear), then b1 loads
        xpose_t(0, 0)
        xpose_q(0, 0)
        xpose_q(0, 1)
        xpose_t(0, 1)
        xpose_q(0, 2)
        xpose_q(0, 3)
        # hint the scheduler to keep b1 loads behind the b0 transposes on
        # the sync FIFO (and off batch-0's HBM bandwidth)
        with tc.tile_wait_until(0.016):
            load_b1("t")
            load_b1("q")
            load_b1("v")

        items = []
        for b in range(B_LOC):
            for qb in range(N_QB):
                for g in range((4 * qb + 4) // 2):
                    items.append((b, qb, g))

        def fillers(n):
            for _ in range(n):
                wps = qk_psum.tile([128, 1024], F32, tag="qk")
                nc.tensor.matmul(wps[:, 0:512], lhsT=junk[:, 0:128],
                                 rhs=junk[:])

        def prep_b1(step):
            if step == 0:
                cast_t(1, 0)
                xpose_t(1, 0)
            elif step == 1:
                cast_q(1, 0)
                xpose_q(1, 0)
            elif step == 2:
                cast_v(1, 0)
            elif step == 3:
                cast_t(1, 1)
                xpose_t(1, 1)
            elif step == 4:
                cast_q(1, 1)
                xpose_q(1, 1)
            elif step == 5:
                cast_q(1, 2)
                xpose_q(1, 2)
            elif step == 6:
                cast_q(1, 3)
                xpose_q(1, 3)
            elif step == 7:
                cast_v(1, 1)

        prep_at = {13: 0, 14: 1, 15: 2, 16: 3, 17: 4, 18: 5, 19: 6, 22: 7}

        state = {}

        def qk_group(b, qb, g):
            s_ps = qk_psum.tile([128, 1024], F32, tag="qk")
            num = num_pool.tile([128, 1024], BF16, tag="num")
            rhs = qtp[b][qb][:].rearrange("d j p -> d p j")
            for j, c in enumerate((2 * g, 2 * g + 1)):
                nc.tensor.matmul(
                    s_ps[:, j * 512:j * 512 + 512],
                    lhsT=ttp[b][c // 8][:, c % 8, :],
                    rhs=rhs,
                    start=True, stop=True,
                )
            nc.scalar.activation(num[:], s_ps[:],
                                 mybir.ActivationFunctionType.Exp,
                                 scale=SCALE)
            nc.vector.tensor_scalar_max(num[:], num[:], 1.0)
            i0 = 2 * g - 4 * qb  # chunk pair is diagonal iff i0 >= 0
            if i0 >= 0:
                nc.vector.tensor_tensor(
                    num[:], num[:],
                    r01p[i0 // 2][:].rearrange("p a c -> p (a c)"),
                    op=Alu.mult)
            st = state.setdefault((b, qb), {"ob": None, "num": {}})
            if st["ob"] is None:
                st["ob"] = [ob_psum.tile([128, 2, 256], F32, tag="ob",
                                         name=f"ob_{b}_{qb}_{h}")
                            for h in range(2)]
            st["num"][g] = num

        def pv_group(b, qb, g):
            st = state[(b, qb)]
            num = st["num"].pop(g)
            for j, c in enumerate((2 * g, 2 * g + 1)):
                s0 = j * 512
                for sub in range(4):
                    ob = st["ob"][sub // 2]
                    nc.tensor.matmul(
                        ob[:, sub % 2, 0:129],
                        lhsT=num[:, s0 + sub * 128:s0 + (sub + 1) * 128],
                        rhs=vaug[b][c // 8][:, c % 8, 0:129],
                        start=(c == 0 and sub % 2 == 0),
                        stop=(c == 4 * qb + 3),
                        skip_group_check=True,
                    )

        def finalize(b, qb):
            st = state.pop((b, qb))
            o_tile = fin_pool.tile([128, 4, 128], F32, tag="fin")
            for h in range(2):
                ob_sb = rec_pool.tile([128, 2, 129], F32, tag="rec")
                nc.vector.tensor_copy(ob_sb[:], st["ob"][h][:, :, 0:129])
                for s2 in range(2):
                    nc.gpsimd.normalize_recip(
                        o_tile[:, 2 * h + s2, :],
                        ob_sb[:, s2, 0:128],
                        ob_sb[:, s2, 128:129])
            nc.sync.dma_start(
                o_ext[b, qb * QB:(qb + 1) * QB, :]
                    .rearrange("(s p) d -> p s d", p=128),
                o_tile[:])

        pending = deque()

        def flush_one():
            b, qb, g = pending.popleft()
            pv_group(b, qb, g)
            if g == (4 * qb + 4) // 2 - 1:
                finalize(b, qb)

        n_items = len(items)
        for idx, it in enumerate(items):
            qk_group(*it)
            if idx == 1:
                fillers(4)
            if idx in prep_at:
                prep_b1(prep_at[idx])
            pending.append(it)
            depth = 2 if idx < n_items - 4 else 1
            while len(pending) > depth:
                flush_one()
        while pending:
            flush_one()

    nc.compile()
    return nc


_NC_CACHE = None


def _get_nc():
    global _NC_CACHE
    if _NC_CACHE is None:
        _NC_CACHE = build_attention_core()
    return _NC_CACHE


def kernel(Q: np.ndarray, T: np.ndarray, V: np.ndarray) -> np.ndarray:
    """Full-input entry point: shard over batch, run 8-core SPMD, gather."""
    from concourse.bass_utils import run_bass_kernel_spmd

    Q = np.ascontiguousarray(np.asarray(Q, dtype=np.float32))
    T = np.ascontiguousarray(np.asarray(T, dtype=np.float32))
    V = np.ascontiguousarray(np.asarray(V, dtype=np.float32))
    assert Q.shape == (B, S, D), Q.shape

    nc = _get_nc()
    in_maps = [
        {
            "Q": Q[i * B_LOC:(i + 1) * B_LOC],
            "T": T[i * B_LOC:(i + 1) * B_LOC],
            "V": V[i * B_LOC:(i + 1) * B_LOC],
        }
        for i in range(N_CORES)
    ]
    res = run_bass_kernel_spmd(nc, in_maps, core_ids=list(range(N_CORES)))
    return np.concatenate([res.results[i]["out"] for i in range(N_CORES)], axis=0)
